# revision 1
# baseline (speedup 1.0000x reference)
"""HeatmapMSELoss Trainium2 kernel.

Computes mean((heatmaps_pred - heatmaps_gt)^2) where heatmaps_gt is an
isotropic 2D gaussian (sigma=1, peak 1) rendered at the projection of each
3D joint into each view.

Key identity: the gaussian separates, gt[h,w] = gy[h] * gx[w], so

  sum_hw (pred - gt)^2 = sum_hw pred^2 - 2 * gy^T (pred @ gx) + (sum gy^2)(sum gx^2)

The 142MB gt tensor is never materialized. Per (b,v,j) slice the device
computes sum(pred^2) (scalar-engine square + accumulate) and
m' = pred^T @ gy (one matmul, PSUM column), then a fused DVE
multiply+reduce against gx. The tiny 1D gaussians (2.2MB total) and the
final scalar combine are done on host in float64.

Sharding: data-parallel over batch, 4 batches per core across 8 cores.
"""

import numpy as np

import concourse.bacc as bacc
import concourse.bass as bass
import concourse.tile as tile
from concourse import mybir
from concourse.bass_utils import run_bass_kernel_spmd

B, V, J, H, W = 32, 4, 17, 128, 128
N_CORES = 8
B_LOC = B // N_CORES          # 4 batches per core
GROUPS = B_LOC * V            # 16 (b,v) groups per core
SLICES = GROUPS * J           # 272 slices per core

_CACHE = {}


GPB = 2                    # (b,v) groups per block
NBLK = GROUPS // GPB       # blocks per core
JB = GPB * J               # joints (slices) per block

# chunk sizes (in slices) over the 272 per-core slices: small chunks at the
# start (fast pipeline ramp: compute starts after a ~1us DMA, not ~3us) and
# at the end (short tail after the last DMA lands)
CHUNKS = [4, 4, 4, 5] + [17] * 14 + [9, 8]
assert sum(CHUNKS) == SLICES


def _build_nc(passes=1, chunks=None, load_bufs=6):
    # Bacc (not raw Bass): its finalize() runs the legalization passes that
    # split multi-wait instructions (matmul can carry at most 1 sync wait).
    nc = bacc.Bacc()
    f32 = mybir.dt.float32
    chunks = list(CHUNKS) if chunks is None else list(chunks)
    nck = len(chunks)
    maxck = max(chunks)

    pred = nc.declare_dram_parameter("pred", [SLICES, H, W], f32, isOutput=False)
    gyt = nc.declare_dram_parameter("gyt", [H, SLICES], f32, isOutput=False)
    gxt = nc.declare_dram_parameter("gxt", [W, SLICES], f32, isOutput=False)
    partials = nc.declare_dram_parameter("partials", [128, 2, nck], f32, isOutput=True)

    with tile.TileContext(nc) as tc:
        with (
            tc.tile_pool(name="consts", bufs=1) as consts,
            tc.tile_pool(name="loads", bufs=load_bufs) as loads,
            tc.tile_pool(name="sq", bufs=2) as sqpool,
            tc.tile_pool(name="prod", bufs=2) as prodpool,
            tc.tile_pool(name="psum", bufs=4, space="PSUM") as psumpool,
            tc.tile_pool(name="outs", bufs=1) as outs,
        ):
            # warm-up ACT so the Square table-set load (~2.7us) overlaps the
            # first pred DMA instead of stalling the first real ACT
            warm = consts.tile([128, 1], f32)
            nc.vector.memset(warm[:], 0.0)
            wsq = consts.tile([128, 1], f32)
            nc.scalar.activation(
                out=wsq[:], in_=warm[:], func=mybir.ActivationFunctionType.Square
            )

            gyt_t = consts.tile([H, SLICES], f32)
            nc.sync.dma_start(out=gyt_t[:], in_=gyt[:, :])
            gxt_t = consts.tile([W, SLICES], f32)
            nc.sync.dma_start(out=gxt_t[:], in_=gxt[:, :])

            outcols = outs.tile([128, 2, nck], f32)

            for _p in range(passes):
                s0 = 0
                for c, csz in enumerate(chunks):
                    t = loads.tile([H, maxck, W], f32, tag="loads")
                    nc.sync.dma_start(
                        out=t[:, :csz, :],
                        in_=pred[s0 : s0 + csz].rearrange("s h w -> h s w"),
                    )

                    # s1: per-partition sum of pred^2 over (s, w)
                    sq = sqpool.tile([H, maxck, W], f32, tag="sq")
                    nc.scalar.activation(
                        out=sq[:, :csz, :],
                        in_=t[:, :csz, :],
                        func=mybir.ActivationFunctionType.Square,
                        accum_out=outcols[:, 0, c : c + 1],
                    )

                    # s2: m'_s = pred_s^T @ gy_s per slice -> psum column
                    ps = psumpool.tile([128, maxck], f32, tag="psum")
                    for sj in range(csz):
                        s = s0 + sj
                        nc.tensor.matmul(
                            ps[:, sj : sj + 1],
                            t[:, sj, :],
                            gyt_t[:, s : s + 1],
                            start=True,
                            stop=True,
                        )
                    # dot with gx, then per-partition sum over slices
                    prod = prodpool.tile([128, maxck], f32, tag="prod")
                    nc.vector.tensor_mul(
                        prod[:, :csz], ps[:, :csz], gxt_t[:, s0 : s0 + csz]
                    )
                    nc.vector.reduce_sum(
                        outcols[:, 1, c : c + 1], prod[:, :csz],
                        axis=mybir.AxisListType.X,
                    )
                    s0 += csz

            nc.sync.dma_start(out=partials[:, :, :], in_=outcols[:])

    nc.finalize()  # Bacc: runs legalization (wait splitting) + regalloc
    return nc


def _gaussians(proj_mats_batch, joints_3d_gt_batch):
    """1D gaussians gy [B,V,J,H], gx [B,V,J,W] in float32 (reference math)."""
    joints = joints_3d_gt_batch.astype(np.float32)
    ones = np.ones(joints.shape[:-1] + (1,), dtype=np.float32)
    joints_h = np.concatenate([joints, ones], axis=-1)  # [B, J, 4]
    proj = np.einsum(
        "bvcd,bjd->bvjc", proj_mats_batch.astype(np.float32), joints_h
    ).astype(np.float32)  # [B, V, J, 3]
    joints_2d = proj[..., :2] / proj[..., 2:3]  # (x, y)
    xs = np.arange(W, dtype=np.float32)
    ys = np.arange(H, dtype=np.float32)
    dx2 = (xs - joints_2d[..., 0, None]) ** 2  # [B,V,J,W]
    dy2 = (ys - joints_2d[..., 1, None]) ** 2  # [B,V,J,H]
    gx = np.exp(-0.5 * dx2).astype(np.float32)
    gy = np.exp(-0.5 * dy2).astype(np.float32)
    return gy, gx


def kernel(heatmaps_pred, proj_mats_batch, joints_3d_gt_batch, joints_3d_valid_batch,
           _profile=None):
    heatmaps_pred = np.ascontiguousarray(np.asarray(heatmaps_pred, dtype=np.float32))
    gy, gx = _gaussians(np.asarray(proj_mats_batch), np.asarray(joints_3d_gt_batch))

    # s3 = sum over slices of (sum_h gy^2) * (sum_w gx^2), exact in f64
    s3 = float(
        ((gy.astype(np.float64) ** 2).sum(-1) * (gx.astype(np.float64) ** 2).sum(-1)).sum()
    )

    if "nc" not in _CACHE:
        _CACHE["nc"] = _build_nc()
    nc = _CACHE["nc"]

    in_maps = []
    for c in range(N_CORES):
        bsl = slice(B_LOC * c, B_LOC * (c + 1))
        # slice order: (b_local, v, j) -> s ; tiles are [H|W, SLICES]
        gyt = np.ascontiguousarray(gy[bsl].reshape(SLICES, H).T)
        gxt = np.ascontiguousarray(gx[bsl].reshape(SLICES, W).T)
        in_maps.append(
            {
                "pred": heatmaps_pred[bsl].reshape(SLICES, H, W),
                "gyt": gyt,
                "gxt": gxt,
            }
        )

    res = run_bass_kernel_spmd(nc, in_maps, core_ids=list(range(N_CORES)))
    if _profile is not None:
        _profile["result"] = res
        _profile["in_maps"] = in_maps

    s1 = 0.0
    s2 = 0.0
    for c in range(N_CORES):
        p = res.results[c]["partials"].astype(np.float64)
        s1 += p[:, 0, :].sum()
        s2 += p[:, 1, :].sum()

    total = s1 - 2.0 * s2 + s3
    return np.float32(total / (B * V * J * H * W))



# revision 8
# speedup vs baseline: 1.0195x; 1.0195x over previous
"""HeatmapMSELoss Trainium2 kernel.

Computes mean((heatmaps_pred - heatmaps_gt)^2) where heatmaps_gt is an
isotropic 2D gaussian (sigma=1, peak 1) rendered at the projection of each
3D joint into each view.

Key identity: the gaussian separates, gt[h,w] = gy[h] * gx[w], so

  sum_hw (pred - gt)^2 = sum_hw pred^2 - 2 * gy^T (pred @ gx) + (sum gy^2)(sum gx^2)

The 142MB gt tensor is never materialized. Per (b,v,j) slice the device
computes sum(pred^2) (scalar-engine square + accumulate) and
m' = pred^T @ gy (one matmul, PSUM column), then a fused DVE
multiply+reduce against gx. The tiny 1D gaussians (2.2MB total) and the
final scalar combine are done on host in float64.

Sharding: data-parallel over batch, 4 batches per core across 8 cores.

Schedule notes (cost-model timeline): the DMA engines are the bottleneck
(pred = 17.8MB/core at 360B/ns ~= 49.5us busy). The pred stream is kept
dense from the first transfer: pred chunks are issued first and the small
gy/gx table (one merged DMA) is slotted in after them. The chunk sizes
taper at the end so the Activation engine (square+accumulate) is caught up
when the last chunk lands, minimizing the post-stream serial tail
(dma-complete sem 900ns + last square + out-DMA preamble).
"""

import numpy as np

import concourse.bacc as bacc
import concourse.bass as bass
import concourse.tile as tile
from concourse import mybir
from concourse.bass_utils import run_bass_kernel_spmd

B, V, J, H, W = 32, 4, 17, 128, 128
N_CORES = 8
B_LOC = B // N_CORES          # 4 batches per core
GROUPS = B_LOC * V            # 16 (b,v) groups per core
SLICES = GROUPS * J           # 272 slices per core

_CACHE = {}


# chunk sizes (in slices) over the 272 per-core slices. Large steady-state
# chunks amortize per-DMA overheads; the taper at the end keeps the ACT
# engine caught up (square of chunk k must finish within the transfer time
# of chunk k+1) so the post-DMA tail is just the final small square.
CHUNKS = [17] * 13 + [11, 9, 7, 7, 6, 6, 5]
assert sum(CHUNKS) == SLICES

# position of the (merged gy|gx) table DMA in the load stream: after this
# many pred-chunk DMAs. The first pred transfer then starts as early as the
# HWDGE pipeline allows; the table transfer fills stream slack later.
G_POS = 2


def _build_nc(chunks=None, g_pos=G_POS, load_bufs=6, fused=False):
    # fused=True (DVE tensor_tensor_reduce) wedges the device under the
    # axon/neuronxcc path despite simulating ~150ns faster — keep unfused.
    # Bacc (not raw Bass): its finalize() runs the legalization passes that
    # split multi-wait instructions (matmul can carry at most 1 sync wait).
    nc = bacc.Bacc()
    f32 = mybir.dt.float32
    chunks = list(CHUNKS) if chunks is None else list(chunks)
    nck = len(chunks)
    maxck = max(chunks)

    pred = nc.declare_dram_parameter("pred", [SLICES, H, W], f32, isOutput=False)
    # merged 1D-gaussian table: cols [0, SLICES) = gy^T, [SLICES, 2*SLICES) = gx^T
    gtab = nc.declare_dram_parameter("gtab", [H, 2 * SLICES], f32, isOutput=False)
    partials = nc.declare_dram_parameter("partials", [128, 2, nck], f32, isOutput=True)

    with tile.TileContext(nc) as tc:
        with (
            tc.tile_pool(name="consts", bufs=1) as consts,
            tc.tile_pool(name="loads", bufs=load_bufs) as loads,
            tc.tile_pool(name="sq", bufs=2) as sqpool,
            tc.tile_pool(name="prod", bufs=2) as prodpool,
            tc.tile_pool(name="psum", bufs=4, space="PSUM") as psumpool,
            tc.tile_pool(name="outs", bufs=1) as outs,
        ):
            # warm-up ACT so the Square table-set load (~1.3us) overlaps the
            # first pred DMA instead of stalling the first real ACT
            warm = consts.tile([128, 1], f32)
            nc.vector.memset(warm[:], 0.0)
            wsq = consts.tile([128, 1], f32)
            nc.scalar.activation(
                out=wsq[:], in_=warm[:], func=mybir.ActivationFunctionType.Square
            )

            gtab_t = consts.tile([H, 2 * SLICES], f32)
            gyt_t = gtab_t[:, :SLICES]
            gxt_t = gtab_t[:, SLICES:]

            outcols = outs.tile([128, 2, nck], f32)

            # DMA order on the queue = transfer order: first g_pos pred
            # chunks, then the table, then the rest (prefetched g_pos ahead).
            # All writes precede their readers in program order so the tile
            # framework emits RAW (not WAR) dependencies.
            starts = np.cumsum([0] + list(chunks))

            def issue_load(c):
                t = loads.tile([H, maxck, W], f32, tag="loads")
                lo, hi = int(starts[c]), int(starts[c + 1])
                nc.sync.dma_start(
                    out=t[:, : hi - lo, :],
                    in_=pred[lo:hi].rearrange("s h w -> h s w"),
                )
                return t

            tiles = {c: issue_load(c) for c in range(min(g_pos, nck))}
            nc.sync.dma_start(out=gtab_t[:], in_=gtab[:, :])

            s0 = 0
            for c, csz in enumerate(chunks):
                if c + g_pos < nck:
                    tiles[c + g_pos] = issue_load(c + g_pos)
                t = tiles.pop(c)

                # s1: per-partition sum of pred^2 over (s, w)
                sq = sqpool.tile([H, maxck, W], f32, tag="sq")
                nc.scalar.activation(
                    out=sq[:, :csz, :],
                    in_=t[:, :csz, :],
                    func=mybir.ActivationFunctionType.Square,
                    accum_out=outcols[:, 0, c : c + 1],
                )

                # s2: m'_s = pred_s^T @ gy_s per slice -> psum column
                ps = psumpool.tile([128, maxck], f32, tag="psum")
                for sj in range(csz):
                    s = s0 + sj
                    nc.tensor.matmul(
                        ps[:, sj : sj + 1],
                        t[:, sj, :],
                        gyt_t[:, s : s + 1],
                        start=True,
                        stop=True,
                    )
                # dot with gx fused with the per-partition sum over slices
                prod = prodpool.tile([128, maxck], f32, tag="prod")
                if fused:
                    nc.vector.tensor_tensor_reduce(
                        out=prod[:, :csz],
                        in0=ps[:, :csz],
                        in1=gxt_t[:, s0 : s0 + csz],
                        scale=1.0,
                        scalar=0.0,
                        op0=mybir.AluOpType.mult,
                        op1=mybir.AluOpType.add,
                        accum_out=outcols[:, 1, c : c + 1],
                    )
                else:
                    nc.vector.tensor_mul(
                        prod[:, :csz], ps[:, :csz], gxt_t[:, s0 : s0 + csz]
                    )
                    nc.vector.reduce_sum(
                        outcols[:, 1, c : c + 1], prod[:, :csz],
                        axis=mybir.AxisListType.X,
                    )
                s0 += csz

            nc.sync.dma_start(out=partials[:, :, :], in_=outcols[:])

    nc.finalize()  # Bacc: runs legalization (wait splitting) + regalloc
    return nc


def _gaussians(proj_mats_batch, joints_3d_gt_batch):
    """1D gaussians gy [B,V,J,H], gx [B,V,J,W] in float32 (reference math)."""
    joints = joints_3d_gt_batch.astype(np.float32)
    ones = np.ones(joints.shape[:-1] + (1,), dtype=np.float32)
    joints_h = np.concatenate([joints, ones], axis=-1)  # [B, J, 4]
    proj = np.einsum(
        "bvcd,bjd->bvjc", proj_mats_batch.astype(np.float32), joints_h
    ).astype(np.float32)  # [B, V, J, 3]
    joints_2d = proj[..., :2] / proj[..., 2:3]  # (x, y)
    xs = np.arange(W, dtype=np.float32)
    ys = np.arange(H, dtype=np.float32)
    dx2 = (xs - joints_2d[..., 0, None]) ** 2  # [B,V,J,W]
    dy2 = (ys - joints_2d[..., 1, None]) ** 2  # [B,V,J,H]
    gx = np.exp(-0.5 * dx2).astype(np.float32)
    gy = np.exp(-0.5 * dy2).astype(np.float32)
    return gy, gx


def kernel(heatmaps_pred, proj_mats_batch, joints_3d_gt_batch, joints_3d_valid_batch,
           _profile=None):
    heatmaps_pred = np.ascontiguousarray(np.asarray(heatmaps_pred, dtype=np.float32))
    gy, gx = _gaussians(np.asarray(proj_mats_batch), np.asarray(joints_3d_gt_batch))

    # s3 = sum over slices of (sum_h gy^2) * (sum_w gx^2), exact in f64
    s3 = float(
        ((gy.astype(np.float64) ** 2).sum(-1) * (gx.astype(np.float64) ** 2).sum(-1)).sum()
    )

    if "nc" not in _CACHE:
        _CACHE["nc"] = _build_nc()
    nc = _CACHE["nc"]

    in_maps = []
    for c in range(N_CORES):
        bsl = slice(B_LOC * c, B_LOC * (c + 1))
        # slice order: (b_local, v, j) -> s ; table cols: gy then gx
        gtab = np.concatenate(
            [gy[bsl].reshape(SLICES, H).T, gx[bsl].reshape(SLICES, W).T], axis=1
        )
        in_maps.append(
            {
                "pred": heatmaps_pred[bsl].reshape(SLICES, H, W),
                "gtab": np.ascontiguousarray(gtab),
            }
        )

    res = run_bass_kernel_spmd(nc, in_maps, core_ids=list(range(N_CORES)))
    if _profile is not None:
        _profile["result"] = res
        _profile["in_maps"] = in_maps

    s1 = 0.0
    s2 = 0.0
    for c in range(N_CORES):
        p = res.results[c]["partials"].astype(np.float64)
        s1 += p[:, 0, :].sum()
        s2 += p[:, 1, :].sum()

    total = s1 - 2.0 * s2 + s3
    return np.float32(total / (B * V * J * H * W))


# revision 26
# speedup vs baseline: 1.0353x; 1.0154x over previous
"""HeatmapMSELoss Trainium2 kernel.

Computes mean((heatmaps_pred - heatmaps_gt)^2) where heatmaps_gt is an
isotropic 2D gaussian (sigma=1, peak 1) rendered at the projection of each
3D joint into each view.

Key identity: the gaussian separates, gt[h,w] = gy[h] * gx[w], so

  sum_hw (pred - gt)^2 = sum_hw pred^2 - 2 * gy^T (pred @ gx) + (sum gy^2)(sum gx^2)

The 142MB gt tensor is never materialized. Per (b,v,j) slice the device
computes sum(pred^2) (scalar-engine square + accumulate) and
m' = pred^T @ gy (one matmul, PSUM column), then a DVE multiply+reduce
against gx. The 1D gaussian tables are generated on device from the ~2KB
of projected 2D centers (iota, subtract, square, Exp); the final scalar
combine (and the pred-independent sum gt^2 term) is done on host in f64.

Sharding: data-parallel over batch, 4 batches per core across 8 cores.

Schedule notes (cost-model timeline): the DMA engines are the bottleneck
(pred = 17.8MB/core at 360B/ns ~= 49.5us busy). The pred stream is kept
dense from the first transfer: pred chunks are issued first (the stream
head is HWDGE-cadence gated) and the tiny centers DMA slots in after
them. The chunk sizes taper at the end so the Activation engine is caught
up when the last chunk lands, and the output DMA is split so only the
last chunk's two columns sit on the critical tail (dma-complete sem
900ns + last square + out-DMA preamble + completion sem).
"""

import numpy as np

import concourse.bacc as bacc
import concourse.bass as bass
import concourse.tile as tile
from concourse import mybir
from concourse.bass_utils import run_bass_kernel_spmd

B, V, J, H, W = 32, 4, 17, 128, 128
N_CORES = 8
B_LOC = B // N_CORES          # 4 batches per core
GROUPS = B_LOC * V            # 16 (b,v) groups per core
SLICES = GROUPS * J           # 272 slices per core

_CACHE = {}


# chunk sizes (in slices) over the 272 per-core slices. Large steady-state
# chunks amortize per-DMA overheads; the taper at the end keeps the ACT
# engine caught up (square of chunk k must finish within the transfer time
# of chunk k+1) so the post-DMA tail is just the final small square.
CHUNKS = [17] * 13 + [11, 9, 7, 7, 6, 6, 5]
assert sum(CHUNKS) == SLICES

# position of the tiny centers DMA in the load stream: after this many
# pred-chunk DMAs (also the pred prefetch depth in program order). The
# first pred transfer then starts as early as the HWDGE pipeline allows.
G_POS = 2


def _build_nc(chunks=None, g_pos=G_POS, load_bufs=6):
    # Note: a fused DVE tensor_tensor_reduce for the gx dot wedges the
    # device under the axon/neuronxcc path despite simulating ~150ns
    # faster, so the mul+reduce stays as two instructions.
    # Bacc (not raw Bass): its finalize() runs the legalization passes that
    # split multi-wait instructions (matmul can carry at most 1 sync wait).
    nc = bacc.Bacc()
    f32 = mybir.dt.float32
    chunks = list(CHUNKS) if chunks is None else list(chunks)
    nck = len(chunks)
    maxck = max(chunks)

    pred = nc.declare_dram_parameter("pred", [SLICES, H, W], f32, isOutput=False)
    # per-slice gaussian centers: cols [0, SLICES) = y, [SLICES, 2*SLICES) = x.
    # The 1D gaussian tables gy[h,s] = exp(-0.5 (h - y_s)^2) (and gx) are
    # generated on device (iota + subtract + square + Exp) so only ~2KB is
    # DMA'd instead of a 278KB table.
    yx = nc.declare_dram_parameter("yx", [1, 2 * SLICES], f32, isOutput=False)
    partials = nc.declare_dram_parameter("partials", [128, nck, 2], f32, isOutput=True)

    with tile.TileContext(nc) as tc:
        with (
            tc.tile_pool(name="consts", bufs=1) as consts,
            tc.tile_pool(name="loads", bufs=load_bufs) as loads,
            tc.tile_pool(name="sq", bufs=2) as sqpool,
            tc.tile_pool(name="prod", bufs=2) as prodpool,
            tc.tile_pool(name="psum", bufs=4, space="PSUM") as psumpool,
            tc.tile_pool(name="outs", bufs=1) as outs,
        ):
            yx_t = consts.tile([1, 2 * SLICES], f32)
            yx_rep = consts.tile([H, 2 * SLICES], f32)
            hio = consts.tile([128, 1], f32)
            dcol = consts.tile([H, 2 * SLICES], f32)
            d2 = consts.tile([H, 2 * SLICES], f32)
            gtab_t = consts.tile([H, 2 * SLICES], f32)
            gyt_t = gtab_t[:, :SLICES]
            gxt_t = gtab_t[:, SLICES:]

            outcols = outs.tile([128, nck, 2], f32)

            # DMA order on the queue = transfer order: first g_pos pred
            # chunks, then the table, then the rest (prefetched g_pos ahead).
            # All writes precede their readers in program order so the tile
            # framework emits RAW (not WAR) dependencies.
            starts = np.cumsum([0] + list(chunks))

            def issue_load(c):
                t = loads.tile([H, maxck, W], f32, tag="loads")
                lo, hi = int(starts[c]), int(starts[c + 1])
                nc.sync.dma_start(
                    out=t[:, : hi - lo, :],
                    in_=pred[lo:hi].rearrange("s h w -> h s w"),
                )
                return t

            # pred chunks first (the stream head is HWDGE-cadence gated, so
            # the tiny centers DMA would cost a full 650ns slot there); the
            # centers slot in after g_pos chunks where transfers are dense.
            # Table generation runs on Pool/DVE/ACT while pred transfers.
            tiles = {c: issue_load(c) for c in range(min(g_pos, nck))}
            nc.sync.dma_start(out=yx_t[:], in_=yx[:, :])
            nc.gpsimd.iota(
                hio[:], [[1, 1]], base=0, channel_multiplier=1,
                allow_small_or_imprecise_dtypes=True,
            )
            nc.gpsimd.partition_broadcast(yx_rep[:], yx_t[:])
            in0, _ = bass.broadcast_tensor_aps(hio[:], yx_rep[:])
            nc.vector.tensor_tensor(
                out=dcol[:], in0=in0, in1=yx_rep[:], op=mybir.AluOpType.subtract
            )
            nc.vector.tensor_mul(d2[:], dcol[:], dcol[:])
            nc.scalar.activation(
                out=gtab_t[:], in_=d2[:],
                func=mybir.ActivationFunctionType.Exp, scale=-0.5,
            )

            s0 = 0
            for c, csz in enumerate(chunks):
                if c + g_pos < nck:
                    tiles[c + g_pos] = issue_load(c + g_pos)
                t = tiles.pop(c)

                # s1: per-partition sum of pred^2 over (s, w)
                sq = sqpool.tile([H, maxck, W], f32, tag="sq")
                nc.scalar.activation(
                    out=sq[:, :csz, :],
                    in_=t[:, :csz, :],
                    func=mybir.ActivationFunctionType.Square,
                    accum_out=outcols[:, c, 0:1],
                )

                # s2: m'_s = pred_s^T @ gy_s per slice -> psum column
                ps = psumpool.tile([128, maxck], f32, tag="psum")
                for sj in range(csz):
                    s = s0 + sj
                    nc.tensor.matmul(
                        ps[:, sj : sj + 1],
                        t[:, sj, :],
                        gyt_t[:, s : s + 1],
                        start=True,
                        stop=True,
                    )
                # dot with gx, then per-partition sum over slices
                prod = prodpool.tile([128, maxck], f32, tag="prod")
                nc.vector.tensor_mul(
                    prod[:, :csz], ps[:, :csz], gxt_t[:, s0 : s0 + csz]
                )
                nc.vector.reduce_sum(
                    outcols[:, c, 1:2], prod[:, :csz],
                    axis=mybir.AxisListType.X,
                )
                s0 += csz

            # split output: the bulk columns only depend on chunks 0..nck-2
            # and overlap the final chunk's square; only the last chunk's two
            # columns (tiny transfer) sit on the critical tail
            nc.sync.dma_start(
                out=partials[:, : nck - 1, :], in_=outcols[:, : nck - 1, :]
            )
            nc.sync.dma_start(
                out=partials[:, nck - 1 :, :], in_=outcols[:, nck - 1 :, :]
            )

    nc.finalize()  # Bacc: runs legalization (wait splitting) + regalloc
    return nc


def _gaussians(proj_mats_batch, joints_3d_gt_batch):
    """1D gaussians gy [B,V,J,H], gx [B,V,J,W] (f32, reference math) plus the
    projected 2D centers joints_2d [B,V,J,2] (x, y)."""
    joints = joints_3d_gt_batch.astype(np.float32)
    ones = np.ones(joints.shape[:-1] + (1,), dtype=np.float32)
    joints_h = np.concatenate([joints, ones], axis=-1)  # [B, J, 4]
    proj = np.einsum(
        "bvcd,bjd->bvjc", proj_mats_batch.astype(np.float32), joints_h
    ).astype(np.float32)  # [B, V, J, 3]
    joints_2d = proj[..., :2] / proj[..., 2:3]  # (x, y)
    xs = np.arange(W, dtype=np.float32)
    ys = np.arange(H, dtype=np.float32)
    dx2 = (xs - joints_2d[..., 0, None]) ** 2  # [B,V,J,W]
    dy2 = (ys - joints_2d[..., 1, None]) ** 2  # [B,V,J,H]
    gx = np.exp(-0.5 * dx2).astype(np.float32)
    gy = np.exp(-0.5 * dy2).astype(np.float32)
    return gy, gx, joints_2d


def kernel(heatmaps_pred, proj_mats_batch, joints_3d_gt_batch, joints_3d_valid_batch,
           _profile=None):
    heatmaps_pred = np.ascontiguousarray(np.asarray(heatmaps_pred, dtype=np.float32))
    gy, gx, joints_2d = _gaussians(
        np.asarray(proj_mats_batch), np.asarray(joints_3d_gt_batch)
    )

    # s3 = sum over slices of (sum_h gy^2) * (sum_w gx^2), exact in f64
    s3 = float(
        ((gy.astype(np.float64) ** 2).sum(-1) * (gx.astype(np.float64) ** 2).sum(-1)).sum()
    )

    if "nc" not in _CACHE:
        _CACHE["nc"] = _build_nc()
    nc = _CACHE["nc"]

    in_maps = []
    for c in range(N_CORES):
        bsl = slice(B_LOC * c, B_LOC * (c + 1))
        # slice order: (b_local, v, j) -> s ; center cols: y then x
        yx = np.concatenate(
            [joints_2d[bsl, ..., 1].reshape(SLICES),
             joints_2d[bsl, ..., 0].reshape(SLICES)]
        ).astype(np.float32)[None, :]
        in_maps.append(
            {
                "pred": heatmaps_pred[bsl].reshape(SLICES, H, W),
                "yx": np.ascontiguousarray(yx),
            }
        )

    res = run_bass_kernel_spmd(nc, in_maps, core_ids=list(range(N_CORES)))
    if _profile is not None:
        _profile["result"] = res
        _profile["in_maps"] = in_maps

    s1 = 0.0
    s2 = 0.0
    for c in range(N_CORES):
        p = res.results[c]["partials"].astype(np.float64)
        s1 += p[:, :, 0].sum()
        s2 += p[:, :, 1].sum()

    total = s1 - 2.0 * s2 + s3
    return np.float32(total / (B * V * J * H * W))
